# revision 1
# baseline (speedup 1.0000x reference)
"""DAHead (dual attention head: PAM + CAM) Trainium2 Bass kernel.

Sharding: 8 cores = (batch b, query-half h); core = 2*b + h.
Each core computes PAM for its 2048-query half of one sample (full key range),
then the CAM Gram partial; Gram partials are summed with a pairwise AllReduce,
after which each core finishes CAM for its half.

Math restructuring (validated exact vs the jax reference, rel-l2 ~5e-8):
  - energy computed transposed, [j, i] layout, so softmax needs no transposes
    anywhere in PAM: P^T comes straight out of exp.
  - no max-subtraction in the PAM softmax (energy range is ~±10; exp is safe
    in fp32); normalization folded into the pam psum eviction via a
    partition-broadcast row.
  - v is computed directly transposed (vT = xf^T @ wv^T), the only form the
    PAM AV matmul needs.
  - attn/N, /C, gamma scalings and the v bias fold into host-precomputed
    per-channel constants (cb, g1, gc).
  - CAM softmax(max-G) == exp(minG-G)/sum: one reduce_min + one fused
    exp+rowsum activation per row block.

Walrus on TRN2 allows only ONE sync wait on (self-loading fp32) matmuls, so
the build keeps every matmul's dependencies to a single semaphore: an
all-engine barrier after setup, a dummy PE "absorber" transpose whenever a
chunk's matmuls would otherwise wait on two engines, and ACT-only PSUM
eviction in phase B so bank-reuse WARs merge with the exp RAW on one sem.
"""

import sys
import numpy as np

sys.path.insert(0, "/opt/trn_rl_repo")

from contextlib import ExitStack

import concourse.bass as bass
import concourse.bacc as bacc
import concourse.tile as tile
from concourse import mybir
from concourse.bass_utils import run_bass_kernel_spmd
from concourse.masks import make_identity

F32 = mybir.dt.float32
AF = mybir.ActivationFunctionType

B, C, H, W = 4, 512, 64, 64
CI = C // 2
N = H * W          # 4096
HN = N // 2        # 2048 queries per core
P = 128
CT = C // P        # 4 channel tiles
QT = CI // P       # 2 q/k channel tiles
JT = N // P        # 32 key tiles
ICH = HN // 512    # 4 query chunks of 512
SC = float(1.0 / np.sqrt(np.float32(C)))
SN = float(1.0 / np.sqrt(np.float32(N)))

_CACHE: dict = {}


def _build_bass():
    nc = bacc.Bacc("TRN2", target_bir_lowering=False, debug=False,
                   num_devices=8)
    xp = nc.declare_dram_parameter("xp", [P, CT, N], F32, isOutput=False)
    wqT = nc.declare_dram_parameter("wqT", [P, CT, CI], F32, isOutput=False)
    wkT = nc.declare_dram_parameter("wkT", [P, CT, CI], F32, isOutput=False)
    wvT = nc.declare_dram_parameter("wvT", [P, CT, C], F32, isOutput=False)
    bqp = nc.declare_dram_parameter("bq", [P, QT], F32, isOutput=False)
    bkp = nc.declare_dram_parameter("bk", [P, QT], F32, isOutput=False)
    cbp = nc.declare_dram_parameter("cb", [P, CT], F32, isOutput=False)
    g1p = nc.declare_dram_parameter("g1", [1, 1], F32, isOutput=False)
    gcp = nc.declare_dram_parameter("gcv", [P, 1], F32, isOutput=False)
    outp = nc.declare_dram_parameter("out", [P, CT, HN], F32, isOutput=True)
    g_in = nc.dram_tensor("g_in", [P, CT, C], F32)
    g_out = nc.dram_tensor("g_out", [P, CT, C], F32)

    with tile.TileContext(nc) as tc, ExitStack() as ctx:
        consts = ctx.enter_context(tc.tile_pool(name="consts", bufs=1))
        sap = ctx.enter_context(tc.tile_pool(name="sap", bufs=1))
        wpool = ctx.enter_context(tc.tile_pool(name="wpool", bufs=1))
        psDummy = ctx.enter_context(tc.tile_pool(name="psDummy", bufs=1, space="PSUM"))

        ident = consts.tile([P, P], F32)
        make_identity(nc, ident)
        ones_col = consts.tile([P, 1], F32)
        nc.vector.memset(ones_col, 1.0)
        cb_sb = consts.tile([P, CT], F32)
        nc.sync.dma_start(out=cb_sb, in_=cbp[:])
        g1_sb = consts.tile([1, 1], F32)
        nc.sync.dma_start(out=g1_sb, in_=g1p[:])
        gc_sb = consts.tile([P, 1], F32)
        nc.sync.dma_start(out=gc_sb, in_=gcp[:])

        wq_sb = wpool.tile([P, CT, CI], F32)
        nc.sync.dma_start(out=wq_sb, in_=wqT[:])
        wk_sb = wpool.tile([P, CT, CI], F32)
        nc.sync.dma_start(out=wk_sb, in_=wkT[:])
        wv_sb = wpool.tile([P, CT, C], F32)
        nc.sync.dma_start(out=wv_sb, in_=wvT[:])
        bq_sb = wpool.tile([P, QT], F32)
        nc.sync.dma_start(out=bq_sb, in_=bqp[:])
        bk_sb = wpool.tile([P, QT], F32)
        nc.sync.dma_start(out=bk_sb, in_=bkp[:])

        sa_sb = sap.tile([P, CT, HN], F32)  # tanh(PAM) result, lives to the end
        dummy_ps = psDummy.tile([P, P], F32)  # absorber target, never read

        # PE touches ident pre-barrier so post-barrier primers wait on SP only
        nc.tensor.transpose(dummy_ps, ident, ident)
        # absorb each weight tensor's dma-queue wait with a 1-wait PE op
        nc.tensor.transpose(dummy_ps, wk_sb[:, 0, 0:P], ident)
        nc.tensor.transpose(dummy_ps, wq_sb[:, 0, 0:P], ident)
        nc.tensor.transpose(dummy_ps, wv_sb[:, 0, 0:P], ident)

        with ExitStack() as ab:
            persist = ab.enter_context(tc.tile_pool(name="persist", bufs=1))
            q_sb = persist.tile([P, QT, HN], F32)
            k_sb = persist.tile([P, QT, N], F32)
            vT_sb = persist.tile([P, JT, C], F32)

            # ---------------- phase A: projections q, k, vT ----------------
            with tc.tile_pool(name="stream", bufs=3) as stream, \
                 tc.tile_pool(name="psA", bufs=4, space="PSUM") as psA:
                for jch in range(8):  # 512-wide column chunks over full N
                    jsl = slice(jch * 512, (jch + 1) * 512)
                    st = stream.tile([P, CT, 512], F32, tag="xstream")
                    for kt in range(CT):
                        nc.sync.dma_start(out=st[:, kt, :], in_=xp[:, kt, jsl])
                    for kt in range(CT):
                        nc.tensor.transpose(dummy_ps, st[:, kt, 0:P], ident)
                    for t in range(QT):
                        kp = psA.tile([P, 512], F32, tag="ps")
                        for kt in range(CT):
                            nc.tensor.matmul(
                                kp, wk_sb[:, kt, t * P:(t + 1) * P], st[:, kt, :],
                                start=(kt == 0), stop=(kt == CT - 1))
                        nc.scalar.activation(k_sb[:, t, jsl], kp, AF.Identity,
                                             bias=bk_sb[:, t:t + 1])
                        if jch < ICH:
                            qp = psA.tile([P, 512], F32, tag="ps")
                            for kt in range(CT):
                                nc.tensor.matmul(
                                    qp, wq_sb[:, kt, t * P:(t + 1) * P], st[:, kt, :],
                                    start=(kt == 0), stop=(kt == CT - 1))
                            nc.scalar.activation(q_sb[:, t, jsl], qp, AF.Identity,
                                                 bias=bq_sb[:, t:t + 1])
                    for nt in range(4):
                        vp = psA.tile([P, 512], F32, tag="ps")
                        for kt in range(CT):
                            nc.tensor.matmul(
                                vp, st[:, kt, nt * P:(nt + 1) * P], wv_sb[:, kt, :],
                                start=(kt == 0), stop=(kt == CT - 1))
                        nc.scalar.activation(vT_sb[:, jch * 4 + nt, :], vp, AF.Copy)

            # ---------------- phase B: PAM attention ----------------
            # absorb phase-A's max ACT tick (vT final evict) in one PE wait
            nc.tensor.transpose(dummy_ps, vT_sb[:, JT - 1, 0:P], ident)
            with tc.tile_pool(name="ptpool", bufs=4) as ptp, \
                 tc.tile_pool(name="sst", bufs=2) as sst, \
                 tc.tile_pool(name="xres", bufs=1) as xres, \
                 tc.tile_pool(name="psCS", bufs=1, space="PSUM") as psS, \
                 tc.tile_pool(name="psE", bufs=2, space="PSUM") as psE, \
                 tc.tile_pool(name="psPam", bufs=1, space="PSUM") as psP:
                for ich in range(ICH):
                    isl = slice(ich * 512, (ich + 1) * 512)
                    xr = xres.tile([P, CT, 512], F32, tag="xr")
                    nc.sync.dma_start(out=xr, in_=xp[:, :, isl])
                    pam = [psP.tile([P, 512], F32, tag=f"pam{t}", name=f"pam{t}_{ich}")
                           for t in range(CT)]
                    cs = psS.tile([1, 512], F32, tag="cs")
                    for jt in range(JT):
                        ep = psE.tile([P, 512], F32, tag="e")
                        for kt in range(QT):
                            nc.tensor.matmul(
                                ep, k_sb[:, kt, jt * P:(jt + 1) * P], q_sb[:, kt, isl],
                                start=(kt == 0), stop=(kt == QT - 1))
                        pt = ptp.tile([P, 512], F32, tag="pt")
                        nc.scalar.activation(pt, ep, AF.Exp, scale=SC)
                        if jt == 0:
                            # absorber: one ACT wait; later group-start matmuls
                            # then only see their single psum-WAR sem
                            nc.tensor.transpose(dummy_ps, pt[:, 0:P], ident)
                        nc.tensor.matmul(cs, ones_col[:, 0:1], pt,
                                         start=(jt == 0), stop=(jt == JT - 1))
                        for ct in range(CT):
                            nc.tensor.matmul(pam[ct], vT_sb[:, jt, ct * P:(ct + 1) * P],
                                             pt, start=(jt == 0), stop=(jt == JT - 1))
                    csb = sst.tile([1, 512], F32, tag="csb")
                    nc.scalar.activation(csb, cs, AF.Copy)
                    inv = sst.tile([1, 512], F32, tag="inv")
                    nc.vector.reciprocal(inv, csb)
                    inv2 = sst.tile([1, 512], F32, tag="inv2")
                    nc.vector.tensor_scalar_mul(inv2, inv, g1_sb[0:1, 0:1])
                    bcs = sst.tile([P, 512], F32, tag="bcs")
                    nc.gpsimd.partition_broadcast(bcs, inv2)
                    for ct in range(CT):
                        pams = sst.tile([P, 512], F32, tag="pams")
                        nc.scalar.activation(pams, pam[ct], AF.Copy)
                        t1 = sst.tile([P, 512], F32, tag="t1")
                        nc.vector.tensor_mul(t1, pams, bcs)
                        t2 = sst.tile([P, 512], F32, tag="t2")
                        nc.vector.tensor_add(t2, t1, xr[:, ct, :])
                        nc.scalar.activation(sa_sb[:, ct, isl], t2, AF.Tanh,
                                             bias=cb_sb[:, ct:ct + 1])

        # ---------------- phase C: CAM ----------------
        # absorb phase-B's max ACT tick (last tanh slice) in one PE wait
        nc.tensor.transpose(dummy_ps, ident, ident)
        nc.tensor.transpose(dummy_ps, sa_sb[:, CT - 1, HN - P:HN], ident)
        with tc.tile_pool(name="phC", bufs=1) as phC, \
             tc.tile_pool(name="stg", bufs=3) as stg, \
             tc.tile_pool(name="psT", bufs=3, space="PSUM") as psT, \
             tc.tile_pool(name="psG", bufs=4, space="PSUM") as psG:
            saT_sb = phC.tile([P, HN // P, C], F32)  # [128, 16, 512]
            for it in range(HN // P):
                for ct in range(CT):
                    tp = psT.tile([P, P], F32, tag="tp")
                    nc.tensor.transpose(tp, sa_sb[:, ct, it * P:(it + 1) * P], ident)
                    nc.scalar.activation(saT_sb[:, it, ct * P:(ct + 1) * P], tp, AF.Copy)
            nc.tensor.transpose(dummy_ps, saT_sb[:, 0, 0:P], ident)
            gp_sb = phC.tile([P, CT, C], F32)
            for ct in range(CT):
                gp = psG.tile([P, C], F32, tag="g")
                for it in range(HN // P):
                    nc.tensor.matmul(gp, saT_sb[:, it, ct * P:(ct + 1) * P],
                                     saT_sb[:, it, :],
                                     start=(it == 0), stop=(it == HN // P - 1))
                nc.scalar.activation(gp_sb[:, ct, :], gp, AF.Copy)
            nc.sync.dma_start(out=g_in[:], in_=gp_sb)
            nc.gpsimd.collective_compute(
                "AllReduce", mybir.AluOpType.add,
                replica_groups=[[0, 1], [2, 3], [4, 5], [6, 7]],
                ins=[g_in[:].opt()], outs=[g_out[:].opt()])
            g2_sb = phC.tile([P, CT, C], F32)
            nc.sync.dma_start(out=g2_sb, in_=g_out[:])
            a_sb = phC.tile([P, CT, C], F32)
            for ct in range(CT):
                m = stg.tile([P, 1], F32, tag="m")
                nc.vector.tensor_reduce(out=m, in_=g2_sb[:, ct, :],
                                        op=mybir.AluOpType.min,
                                        axis=mybir.AxisListType.X)
                msc = stg.tile([P, 1], F32, tag="msc")
                nc.vector.tensor_scalar_mul(msc, m, SN)
                s = stg.tile([P, 1], F32, tag="s")
                e = stg.tile([P, C], F32, tag="ec")
                nc.scalar.activation(e, g2_sb[:, ct, :], AF.Exp,
                                     bias=msc, scale=-SN, accum_out=s)
                invc = stg.tile([P, 1], F32, tag="invc")
                nc.vector.reciprocal(invc, s)
                nc.scalar.activation(a_sb[:, ct, :], e, AF.Identity, scale=invc)
            aT_sb = phC.tile([P, CT, C], F32)
            for ct in range(CT):
                for dt in range(CT):
                    tp = psT.tile([P, P], F32, tag="tp")
                    nc.tensor.transpose(tp, a_sb[:, ct, dt * P:(dt + 1) * P], ident)
                    nc.scalar.activation(aT_sb[:, dt, ct * P:(ct + 1) * P], tp, AF.Copy)
            for ct in range(CT):
                for ich in range(ICH):
                    isl = slice(ich * 512, (ich + 1) * 512)
                    cp = psG.tile([P, 512], F32, tag="g")
                    for dt in range(CT):
                        nc.tensor.matmul(cp, aT_sb[:, dt, ct * P:(ct + 1) * P],
                                         sa_sb[:, dt, isl],
                                         start=(dt == 0), stop=(dt == CT - 1))
                    t1 = stg.tile([P, 512], F32, tag="o1")
                    nc.vector.tensor_scalar_mul(t1, cp, gc_sb[:, 0:1])
                    o = stg.tile([P, 512], F32, tag="o2")
                    nc.vector.tensor_add(o, t1, sa_sb[:, ct, isl])
                    nc.sync.dma_start(out=outp[:, ct, isl], in_=o)
    nc.compile()
    return nc


def _get_nc():
    if "nc" not in _CACHE:
        _CACHE["nc"] = _build_bass()
    return _CACHE["nc"]


def _part(a2d, nt):
    """[nt*128, F] -> [128, nt, F] contiguous (partition-major tiles)."""
    f = a2d.shape[1]
    return np.ascontiguousarray(
        a2d.reshape(nt, P, f).transpose(1, 0, 2).astype(np.float32))


def _in_maps(x, wq, bq, wk, bk, wv, bv, gamma_pam, gamma_cam):
    gp = float(np.asarray(gamma_pam).reshape(-1)[0])
    gc = float(np.asarray(gamma_cam).reshape(-1)[0])
    wq_a = _part(np.asarray(wq, np.float32).T, CT)
    wk_a = _part(np.asarray(wk, np.float32).T, CT)
    wv_a = _part(np.asarray(wv, np.float32).T, CT)
    bq_a = np.ascontiguousarray(np.asarray(bq, np.float32).reshape(QT, P).T)
    bk_a = np.ascontiguousarray(np.asarray(bk, np.float32).reshape(QT, P).T)
    cb_a = np.ascontiguousarray(
        (gp * np.asarray(bv, np.float32) / N).reshape(CT, P).T)
    g1_a = np.full((1, 1), gp / N, np.float32)
    gc_a = np.full((P, 1), gc / C, np.float32)
    maps = []
    for core in range(8):
        b, h = core // 2, core % 2
        xr = np.asarray(x, np.float32)[b].reshape(C, N)
        xperm = np.concatenate(
            [xr[:, h * HN:(h + 1) * HN], xr[:, (1 - h) * HN:(2 - h) * HN]], axis=1)
        maps.append({
            "xp": _part(xperm, CT), "wqT": wq_a, "wkT": wk_a, "wvT": wv_a,
            "bq": bq_a, "bk": bk_a, "cb": cb_a, "g1": g1_a, "gcv": gc_a,
        })
    return maps


def _run(in_maps, **kw):
    return run_bass_kernel_spmd(_get_nc(), in_maps, list(range(8)), **kw)


def kernel(**inputs) -> np.ndarray:
    maps = _in_maps(**inputs)
    res = _run(maps).results
    out = np.zeros((B, C, N), np.float32)
    for core in range(8):
        b, h = core // 2, core % 2
        o = np.asarray(res[core]["out"])  # [128, CT, HN]
        out[b][:, h * HN:(h + 1) * HN] = o.transpose(1, 0, 2).reshape(C, HN)
    return out.reshape(B, C, H, W)



# revision 4
# speedup vs baseline: 1.4310x; 1.4310x over previous
"""DAHead (dual attention head: PAM + CAM) Trainium2 Bass kernel.

Sharding: 8 cores = (batch b, query-half h); core = 2*b + h.
Each core computes PAM for its 2048-query half of one sample (full key range),
then the CAM Gram partial; Gram partials are summed with a pairwise AllReduce,
after which each core finishes CAM for its half.

Math restructuring (validated vs the jax reference):
  - energy computed transposed, [j, i] layout, so softmax needs no transposes
    anywhere in PAM: P^T comes straight out of exp.
  - no max-subtraction in the PAM softmax (energy range is ~±10; exp is safe
    in fp32); normalization folded into the pam psum eviction via a
    partition-broadcast row.
  - v is computed directly transposed (vT = xf^T @ wv^T), the only form the
    PAM AV matmul needs.
  - attn/N, /C, gamma scalings and the v bias fold into host-precomputed
    per-channel constants (cb, g1, gc).
  - CAM softmax(max-G) == exp(minG-G)/sum: one reduce_min + one fused
    exp+rowsum activation per row block.

All matmul operands are bf16 (1 cycle/row on the PE vs 4 for fp32);
accumulation stays fp32 in PSUM, and the residual path (x, biases, softmax
normalization, final residual add) stays fp32.  x and the weights are
converted to bf16 on the host, so the kernel does no on-core conversions.
"""

import sys
import numpy as np
import ml_dtypes

sys.path.insert(0, "/opt/trn_rl_repo")

from contextlib import ExitStack

import concourse.bass as bass
import concourse.bacc as bacc
import concourse.tile as tile
from concourse import mybir
from concourse.bass_utils import run_bass_kernel_spmd
from concourse.masks import make_identity

F32 = mybir.dt.float32
BF16 = mybir.dt.bfloat16
AF = mybir.ActivationFunctionType
NPBF = ml_dtypes.bfloat16

B, C, H, W = 4, 512, 64, 64
CI = C // 2
N = H * W          # 4096
HN = N // 2        # 2048 queries per core
P = 128
CT = C // P        # 4 channel tiles
QT = CI // P       # 2 q/k channel tiles
JT = N // P        # 32 key tiles
ICH = HN // 512    # 4 query chunks of 512
SC = float(1.0 / np.sqrt(np.float32(C)))
SN = float(1.0 / np.sqrt(np.float32(N)))

_CACHE: dict = {}


def _build_bass(sim=False):
    nc = bacc.Bacc("TRN2", target_bir_lowering=False, debug=False,
                   num_devices=8)
    xb = nc.declare_dram_parameter("xb", [P, CT, N], BF16, isOutput=False)
    xr32 = nc.declare_dram_parameter("xr32", [P, CT, HN], F32, isOutput=False)
    wqT = nc.declare_dram_parameter("wqT", [P, CT, CI], BF16, isOutput=False)
    wkT = nc.declare_dram_parameter("wkT", [P, CT, CI], BF16, isOutput=False)
    wvT = nc.declare_dram_parameter("wvT", [P, CT, C], BF16, isOutput=False)
    bqp = nc.declare_dram_parameter("bq", [P, QT], F32, isOutput=False)
    bkp = nc.declare_dram_parameter("bk", [P, QT], F32, isOutput=False)
    cbp = nc.declare_dram_parameter("cb", [P, CT], F32, isOutput=False)
    g1p = nc.declare_dram_parameter("g1", [1, 1], F32, isOutput=False)
    gcp = nc.declare_dram_parameter("gcv", [P, 1], F32, isOutput=False)
    outp = nc.declare_dram_parameter("out", [P, CT, HN], F32, isOutput=True)
    g_in = nc.dram_tensor("g_in", [P, CT, C], F32)
    g_out = nc.dram_tensor("g_out", [P, CT, C], F32)

    with tile.TileContext(nc) as tc, ExitStack() as ctx:
        consts = ctx.enter_context(tc.tile_pool(name="consts", bufs=1))
        sap = ctx.enter_context(tc.tile_pool(name="sap", bufs=1))
        wpool = ctx.enter_context(tc.tile_pool(name="wpool", bufs=1))
        psDummy = ctx.enter_context(tc.tile_pool(name="psDummy", bufs=1, space="PSUM"))

        ident = consts.tile([P, P], BF16)
        make_identity(nc, ident)
        ones_col = consts.tile([P, 1], BF16)
        nc.vector.memset(ones_col, 1.0)
        cb_sb = consts.tile([P, CT], F32)
        nc.sync.dma_start(out=cb_sb, in_=cbp[:])
        g1_sb = consts.tile([1, 1], F32)
        nc.sync.dma_start(out=g1_sb, in_=g1p[:])
        gc_sb = consts.tile([P, 1], F32)
        nc.sync.dma_start(out=gc_sb, in_=gcp[:])

        wq_sb = wpool.tile([P, CT, CI], BF16)
        nc.sync.dma_start(out=wq_sb, in_=wqT[:])
        wk_sb = wpool.tile([P, CT, CI], BF16)
        nc.sync.dma_start(out=wk_sb, in_=wkT[:])
        wv_sb = wpool.tile([P, CT, C], BF16)
        nc.sync.dma_start(out=wv_sb, in_=wvT[:])
        bq_sb = wpool.tile([P, QT], F32)
        nc.sync.dma_start(out=bq_sb, in_=bqp[:])
        bk_sb = wpool.tile([P, QT], F32)
        nc.sync.dma_start(out=bk_sb, in_=bkp[:])

        sa_sb = sap.tile([P, CT, HN], BF16)  # tanh(PAM) result, lives to the end
        dummy_ps = psDummy.tile([P, P], BF16)  # absorber target, never read

        # PE touches ident pre-barrier so post-barrier primers wait on SP only
        nc.tensor.transpose(dummy_ps, ident, ident)
        # absorb each weight tensor's dma-queue wait with a 1-wait PE op
        nc.tensor.transpose(dummy_ps, wk_sb[:, 0, 0:P], ident)
        nc.tensor.transpose(dummy_ps, wq_sb[:, 0, 0:P], ident)
        nc.tensor.transpose(dummy_ps, wv_sb[:, 0, 0:P], ident)

        with ExitStack() as ab:
            persist = ab.enter_context(tc.tile_pool(name="persist", bufs=1))
            q_sb = persist.tile([P, QT, HN], BF16)
            k_sb = persist.tile([P, QT, N], BF16)
            vT_sb = persist.tile([P, JT, C], BF16)

            # ---------------- phase A: projections q, k, vT ----------------
            with tc.tile_pool(name="stream", bufs=3) as stream, \
                 tc.tile_pool(name="psA", bufs=4, space="PSUM") as psA:
                for jch in range(8):  # 512-wide column chunks over full N
                    jsl = slice(jch * 512, (jch + 1) * 512)
                    st = stream.tile([P, CT, 512], BF16, tag="xstream")
                    for kt in range(CT):
                        nc.sync.dma_start(out=st[:, kt, :], in_=xb[:, kt, jsl])
                    for kt in range(CT):
                        nc.tensor.transpose(dummy_ps, st[:, kt, 0:P], ident)
                    for t in range(QT):
                        kp = psA.tile([P, 512], F32, tag="ps")
                        for kt in range(CT):
                            nc.tensor.matmul(
                                kp, wk_sb[:, kt, t * P:(t + 1) * P], st[:, kt, :],
                                start=(kt == 0), stop=(kt == CT - 1))
                        nc.scalar.activation(k_sb[:, t, jsl], kp, AF.Identity,
                                             bias=bk_sb[:, t:t + 1])
                        if jch < ICH:
                            qp = psA.tile([P, 512], F32, tag="ps")
                            for kt in range(CT):
                                nc.tensor.matmul(
                                    qp, wq_sb[:, kt, t * P:(t + 1) * P], st[:, kt, :],
                                    start=(kt == 0), stop=(kt == CT - 1))
                            nc.scalar.activation(q_sb[:, t, jsl], qp, AF.Identity,
                                                 bias=bq_sb[:, t:t + 1])
                    for nt in range(4):
                        vp = psA.tile([P, 512], F32, tag="ps")
                        for kt in range(CT):
                            nc.tensor.matmul(
                                vp, st[:, kt, nt * P:(nt + 1) * P], wv_sb[:, kt, :],
                                start=(kt == 0), stop=(kt == CT - 1))
                        nc.scalar.activation(vT_sb[:, jch * 4 + nt, :], vp, AF.Copy)

            # ---------------- phase B: PAM attention ----------------
            # absorb phase-A's max ACT tick (vT final evict) in one PE wait
            nc.tensor.transpose(dummy_ps, vT_sb[:, JT - 1, 0:P], ident)
            with tc.tile_pool(name="ptpool", bufs=4) as ptp, \
                 tc.tile_pool(name="sst", bufs=2) as sst, \
                 tc.tile_pool(name="xres", bufs=1) as xres, \
                 tc.tile_pool(name="psCS", bufs=1, space="PSUM") as psS, \
                 tc.tile_pool(name="psE", bufs=2, space="PSUM") as psE, \
                 tc.tile_pool(name="psPam", bufs=1, space="PSUM") as psP:
                for ich in range(ICH):
                    isl = slice(ich * 512, (ich + 1) * 512)
                    xr = xres.tile([P, CT, 512], F32, tag="xr")
                    nc.sync.dma_start(out=xr, in_=xr32[:, :, isl])
                    pam = [psP.tile([P, 512], F32, tag=f"pam{t}", name=f"pam{t}_{ich}")
                           for t in range(CT)]
                    cs = psS.tile([1, 512], F32, tag="cs")
                    for jt in range(JT):
                        ep = psE.tile([P, 512], F32, tag="e")
                        for kt in range(QT):
                            nc.tensor.matmul(
                                ep, k_sb[:, kt, jt * P:(jt + 1) * P], q_sb[:, kt, isl],
                                start=(kt == 0), stop=(kt == QT - 1))
                        pt = ptp.tile([P, 512], BF16, tag="pt")
                        nc.scalar.activation(pt, ep, AF.Exp, scale=SC)
                        if jt == 0:
                            # absorber: one ACT wait; later group-start matmuls
                            # then only see their single psum-WAR sem
                            nc.tensor.transpose(dummy_ps, pt[:, 0:P], ident)
                        nc.tensor.matmul(cs, ones_col[:, 0:1], pt,
                                         start=(jt == 0), stop=(jt == JT - 1))
                        for ct in range(CT):
                            nc.tensor.matmul(pam[ct], vT_sb[:, jt, ct * P:(ct + 1) * P],
                                             pt, start=(jt == 0), stop=(jt == JT - 1))
                    csb = sst.tile([1, 512], F32, tag="csb")
                    nc.scalar.activation(csb, cs, AF.Copy)
                    inv = sst.tile([1, 512], F32, tag="inv")
                    nc.vector.reciprocal(inv, csb)
                    inv2 = sst.tile([1, 512], F32, tag="inv2")
                    nc.vector.tensor_scalar_mul(inv2, inv, g1_sb[0:1, 0:1])
                    bcs = sst.tile([P, 512], F32, tag="bcs")
                    nc.gpsimd.partition_broadcast(bcs, inv2)
                    for ct in range(CT):
                        pams = sst.tile([P, 512], F32, tag="pams")
                        nc.scalar.activation(pams, pam[ct], AF.Copy)
                        t1 = sst.tile([P, 512], F32, tag="t1")
                        nc.vector.tensor_mul(t1, pams, bcs)
                        t2 = sst.tile([P, 512], F32, tag="t2")
                        nc.vector.tensor_add(t2, t1, xr[:, ct, :])
                        nc.scalar.activation(sa_sb[:, ct, isl], t2, AF.Tanh,
                                             bias=cb_sb[:, ct:ct + 1])

        # ---------------- phase C: CAM ----------------
        # absorb phase-B's max ACT tick (last tanh slice) in one PE wait
        nc.tensor.transpose(dummy_ps, ident, ident)
        nc.tensor.transpose(dummy_ps, sa_sb[:, CT - 1, HN - P:HN], ident)
        with tc.tile_pool(name="phC", bufs=1) as phC, \
             tc.tile_pool(name="stg", bufs=3) as stg, \
             tc.tile_pool(name="psT", bufs=3, space="PSUM") as psT, \
             tc.tile_pool(name="psG", bufs=4, space="PSUM") as psG:
            saT_sb = phC.tile([P, HN // P, C], BF16)  # [128, 16, 512]
            for it in range(HN // P):
                for ct in range(CT):
                    tp = psT.tile([P, P], BF16, tag="tp")
                    nc.tensor.transpose(tp, sa_sb[:, ct, it * P:(it + 1) * P], ident)
                    nc.scalar.activation(saT_sb[:, it, ct * P:(ct + 1) * P], tp, AF.Copy)
            nc.tensor.transpose(dummy_ps, saT_sb[:, 0, 0:P], ident)
            gp_sb = phC.tile([P, CT, C], F32)
            for ct in range(CT):
                gp = psG.tile([P, C], F32, tag="g")
                for it in range(HN // P):
                    nc.tensor.matmul(gp, saT_sb[:, it, ct * P:(ct + 1) * P],
                                     saT_sb[:, it, :],
                                     start=(it == 0), stop=(it == HN // P - 1))
                nc.scalar.activation(gp_sb[:, ct, :], gp, AF.Copy)
            nc.sync.dma_start(out=g_in[:], in_=gp_sb)
            if sim:
                # timing-only stand-in for the pairwise AllReduce
                nc.sync.dma_start(out=g_out[:], in_=g_in[:])
            else:
                nc.gpsimd.collective_compute(
                    "AllReduce", mybir.AluOpType.add,
                    replica_groups=[[0, 1], [2, 3], [4, 5], [6, 7]],
                    ins=[g_in[:].opt()], outs=[g_out[:].opt()])
            g2_sb = phC.tile([P, CT, C], F32)
            nc.sync.dma_start(out=g2_sb, in_=g_out[:])
            a_sb = phC.tile([P, CT, C], BF16)
            for ct in range(CT):
                m = stg.tile([P, 1], F32, tag="m")
                nc.vector.tensor_reduce(out=m, in_=g2_sb[:, ct, :],
                                        op=mybir.AluOpType.min,
                                        axis=mybir.AxisListType.X)
                msc = stg.tile([P, 1], F32, tag="msc")
                nc.vector.tensor_scalar_mul(msc, m, SN)
                s = stg.tile([P, 1], F32, tag="s")
                e = stg.tile([P, C], F32, tag="ec")
                nc.scalar.activation(e, g2_sb[:, ct, :], AF.Exp,
                                     bias=msc, scale=-SN, accum_out=s)
                invc = stg.tile([P, 1], F32, tag="invc")
                nc.vector.reciprocal(invc, s)
                nc.scalar.activation(a_sb[:, ct, :], e, AF.Identity, scale=invc)
            aT_sb = phC.tile([P, CT, C], BF16)
            for ct in range(CT):
                for dt in range(CT):
                    tp = psT.tile([P, P], BF16, tag="tp")
                    nc.tensor.transpose(tp, a_sb[:, ct, dt * P:(dt + 1) * P], ident)
                    nc.scalar.activation(aT_sb[:, dt, ct * P:(ct + 1) * P], tp, AF.Copy)
            for ct in range(CT):
                for ich in range(ICH):
                    isl = slice(ich * 512, (ich + 1) * 512)
                    cp = psG.tile([P, 512], F32, tag="g")
                    for dt in range(CT):
                        nc.tensor.matmul(cp, aT_sb[:, dt, ct * P:(ct + 1) * P],
                                         sa_sb[:, dt, isl],
                                         start=(dt == 0), stop=(dt == CT - 1))
                    t1 = stg.tile([P, 512], F32, tag="o1")
                    nc.vector.tensor_scalar_mul(t1, cp, gc_sb[:, 0:1])
                    o = stg.tile([P, 512], F32, tag="o2")
                    nc.vector.tensor_add(o, t1, sa_sb[:, ct, isl])
                    nc.sync.dma_start(out=outp[:, ct, isl], in_=o)
    nc.compile()
    return nc


def _get_nc():
    if "nc" not in _CACHE:
        _CACHE["nc"] = _build_bass()
    return _CACHE["nc"]


def _part(a2d, nt, dtype=np.float32):
    """[nt*128, F] -> [128, nt, F] contiguous (partition-major tiles)."""
    f = a2d.shape[1]
    return np.ascontiguousarray(
        a2d.reshape(nt, P, f).transpose(1, 0, 2).astype(dtype))


def _in_maps(x, wq, bq, wk, bk, wv, bv, gamma_pam, gamma_cam):
    gp = float(np.asarray(gamma_pam).reshape(-1)[0])
    gc = float(np.asarray(gamma_cam).reshape(-1)[0])
    wq_a = _part(np.asarray(wq, np.float32).T, CT, NPBF)
    wk_a = _part(np.asarray(wk, np.float32).T, CT, NPBF)
    wv_a = _part(np.asarray(wv, np.float32).T, CT, NPBF)
    bq_a = np.ascontiguousarray(np.asarray(bq, np.float32).reshape(QT, P).T)
    bk_a = np.ascontiguousarray(np.asarray(bk, np.float32).reshape(QT, P).T)
    cb_a = np.ascontiguousarray(
        (gp * np.asarray(bv, np.float32) / N).reshape(CT, P).T)
    g1_a = np.full((1, 1), gp / N, np.float32)
    gc_a = np.full((P, 1), gc / C, np.float32)
    maps = []
    for core in range(8):
        b, h = core // 2, core % 2
        xr = np.asarray(x, np.float32)[b].reshape(C, N)
        xperm = np.concatenate(
            [xr[:, h * HN:(h + 1) * HN], xr[:, (1 - h) * HN:(2 - h) * HN]], axis=1)
        maps.append({
            "xb": _part(xperm, CT, NPBF),
            "xr32": _part(xperm[:, :HN], CT),
            "wqT": wq_a, "wkT": wk_a, "wvT": wv_a,
            "bq": bq_a, "bk": bk_a, "cb": cb_a, "g1": g1_a, "gcv": gc_a,
        })
    return maps


def _run(in_maps, **kw):
    return run_bass_kernel_spmd(_get_nc(), in_maps, list(range(8)), **kw)


def kernel(**inputs) -> np.ndarray:
    maps = _in_maps(**inputs)
    res = _run(maps).results
    out = np.zeros((B, C, N), np.float32)
    for core in range(8):
        b, h = core // 2, core % 2
        o = np.asarray(res[core]["out"])  # [128, CT, HN]
        out[b][:, h * HN:(h + 1) * HN] = o.transpose(1, 0, 2).reshape(C, HN)
    return out.reshape(B, C, H, W)


# revision 33
# speedup vs baseline: 13966.9498x; 9760.0958x over previous
"""DAHead (dual attention head: PAM + CAM) Trainium2 Bass kernel.

Sharding: 8 cores = (batch b, query-half h); core = 2*b + h.
Each core computes PAM for its 2048-query half of one sample (full key range),
then the CAM Gram partial; Gram partials are summed with a pairwise AllReduce,
after which each core finishes CAM for its half.

Math restructuring (validated vs the jax reference):
  - energy computed transposed, [j, i] layout, so softmax needs no transposes
    anywhere in PAM: P^T comes straight out of exp.
  - no max-subtraction in the PAM softmax (energy range is ~±10; exp is safe
    in fp32); normalization folded into the pam psum eviction via a
    partition-broadcast row.
  - v is computed directly transposed (vT = xf^T @ wv^T), the only form the
    PAM AV matmul needs.
  - attn/N, /C, gamma scalings and the v bias fold into host-precomputed
    per-channel constants (cb, g1, gc).
  - CAM softmax(max-G) == exp(minG-G)/sum: one reduce_min + one fused
    exp+rowsum activation per row block.

All matmul operands are bf16 (1 cycle/row on the PE vs 4 for fp32);
accumulation stays fp32 in PSUM, and the residual path (x, biases, softmax
normalization, final residual add) stays fp32.  x and the weights are
converted to bf16 on the host, so the kernel does no on-core conversions.
"""

import sys
import numpy as np
import ml_dtypes

sys.path.insert(0, "/opt/trn_rl_repo")

from contextlib import ExitStack

import concourse.bass as bass
import concourse.bacc as bacc
import concourse.tile as tile
from concourse import mybir
from concourse.bass_utils import run_bass_kernel_spmd
from concourse.masks import make_identity

F32 = mybir.dt.float32
BF16 = mybir.dt.bfloat16
FP8 = mybir.dt.float8e4
DR = mybir.MatmulPerfMode.DoubleRow
AF = mybir.ActivationFunctionType
NPBF = ml_dtypes.bfloat16
ESH = 5.0  # softmax exp shift: pt = exp(logit - ESH); cancels in pam/cs ratio

B, C, H, W = 4, 512, 64, 64
CI = C // 2
N = H * W          # 4096
HN = N // 2        # 2048 queries per core
P = 128
CT = C // P        # 4 channel tiles
QT = CI // P       # 2 q/k channel tiles
JT = N // P        # 32 key tiles
ICH = HN // 512    # 4 query chunks of 512
SC = float(1.0 / np.sqrt(np.float32(C)))
SN = float(1.0 / np.sqrt(np.float32(N)))

_CACHE: dict = {}


def _build_bass(sim=False):
    nc = bacc.Bacc("TRN2", target_bir_lowering=False, debug=False,
                   num_devices=8)
    xb = nc.declare_dram_parameter("xb", [P, CT, N], BF16, isOutput=False)
    xr32 = nc.declare_dram_parameter("xr32", [P, CT, HN], F32, isOutput=False)
    wqT = nc.declare_dram_parameter("wqT", [P, CT, CI], BF16, isOutput=False)
    wkT = nc.declare_dram_parameter("wkT", [P, CT, CI], BF16, isOutput=False)
    wvT = nc.declare_dram_parameter("wvT", [P, CT, C], BF16, isOutput=False)
    bqp = nc.declare_dram_parameter("bq", [P, QT], F32, isOutput=False)
    bkp = nc.declare_dram_parameter("bk", [P, QT], F32, isOutput=False)
    cbp = nc.declare_dram_parameter("cb", [P, CT], F32, isOutput=False)
    g1p = nc.declare_dram_parameter("g1", [1, 1], F32, isOutput=False)
    gcp = nc.declare_dram_parameter("gcv", [P, 1], F32, isOutput=False)
    outp = nc.declare_dram_parameter("out", [P, CT, HN], F32, isOutput=True)
    g_ins = [nc.dram_tensor(f"g_in{ct}", [P, C], F32) for ct in range(CT)]
    g_outs = [nc.dram_tensor(f"g_out{ct}", [P, C], F32) for ct in range(CT)]

    with tile.TileContext(nc) as tc, ExitStack() as ctx:
        consts = ctx.enter_context(tc.tile_pool(name="consts", bufs=1))
        sap = ctx.enter_context(tc.tile_pool(name="sap", bufs=1))
        wpool = ctx.enter_context(tc.tile_pool(name="wpool", bufs=1))
        psTr = ctx.enter_context(tc.tile_pool(name="psTr", bufs=1, space="PSUM"))

        ident = consts.tile([P, P], BF16)
        make_identity(nc, ident)
        ones2 = consts.tile([P, 2, 16], FP8)  # 16-wide: DR ldweights needs step%16==0
        nc.vector.memset(ones2, 1.0)
        esh_sb = consts.tile([P, 1], F32)
        nc.vector.memset(esh_sb, -ESH)
        cb_sb = consts.tile([P, CT], F32)
        nc.sync.dma_start(out=cb_sb, in_=cbp[:])
        g1_sb = consts.tile([1, 1], F32)
        nc.sync.dma_start(out=g1_sb, in_=g1p[:])
        gc_sb = consts.tile([P, 1], F32)
        nc.sync.dma_start(out=gc_sb, in_=gcp[:])

        wq_sb = wpool.tile([P, CT, CI], BF16)
        nc.sync.dma_start(out=wq_sb, in_=wqT[:])
        wk_sb = wpool.tile([P, CT, CI], BF16)
        nc.sync.dma_start(out=wk_sb, in_=wkT[:])
        wv_sb = wpool.tile([P, CT, C], BF16)
        nc.sync.dma_start(out=wv_sb, in_=wvT[:])
        bq_sb = wpool.tile([P, QT], F32)
        nc.sync.dma_start(out=bq_sb, in_=bqp[:])
        bk_sb = wpool.tile([P, QT], F32)
        nc.sync.dma_start(out=bk_sb, in_=bkp[:])

        sa_sb = sap.tile([P, CT, HN], BF16)  # tanh(PAM) result, lives to the end
        saT_sb = sap.tile([P, HN // P, C], BF16)  # sa^T, built during phase B

        with ExitStack() as ab:
            persist = ab.enter_context(tc.tile_pool(name="persist", bufs=1))
            q_sb = persist.tile([P, QT, HN], FP8)
            k_sb = persist.tile([P, QT, N], FP8)
            vT_sb = persist.tile([P, JT, C], FP8)

            # ---------------- phase A: projections q, k, vT ----------------
            with tc.tile_pool(name="stream", bufs=3) as stream, \
                 tc.tile_pool(name="psA", bufs=4, space="PSUM") as psA:
                for jch in range(8):  # 512-wide column chunks over full N
                    jsl = slice(jch * 512, (jch + 1) * 512)
                    st = stream.tile([P, CT, 512], BF16, tag="xstream")
                    for kt in range(CT):
                        nc.sync.dma_start(out=st[:, kt, :], in_=xb[:, kt, jsl])
                    for t in range(QT):
                        kp = psA.tile([P, 512], F32, tag="ps")
                        for kt in range(CT):
                            nc.tensor.matmul(
                                kp, wk_sb[:, kt, t * P:(t + 1) * P], st[:, kt, :],
                                start=(kt == 0), stop=(kt == CT - 1))
                        nc.vector.tensor_scalar_add(k_sb[:, t, jsl], kp,
                                                    bk_sb[:, t:t + 1])
                        if jch < ICH:
                            qp = psA.tile([P, 512], F32, tag="ps")
                            for kt in range(CT):
                                nc.tensor.matmul(
                                    qp, wq_sb[:, kt, t * P:(t + 1) * P], st[:, kt, :],
                                    start=(kt == 0), stop=(kt == CT - 1))
                            nc.vector.tensor_scalar_add(q_sb[:, t, jsl], qp,
                                                        bq_sb[:, t:t + 1])
                    for nt in range(4):
                        vp = psA.tile([P, 512], F32, tag="ps")
                        for kt in range(CT):
                            nc.tensor.matmul(
                                vp, st[:, kt, nt * P:(nt + 1) * P], wv_sb[:, kt, :],
                                start=(kt == 0), stop=(kt == CT - 1))
                        nc.vector.tensor_copy(vT_sb[:, jch * 4 + nt, :], vp)

            # ---------------- phase B: PAM attention ----------------
            with tc.tile_pool(name="ptpool", bufs=4) as ptp, \
                 tc.tile_pool(name="sst", bufs=2) as sst, \
                 tc.tile_pool(name="xres", bufs=1) as xres, \
                 tc.tile_pool(name="psCS", bufs=1, space="PSUM") as psS, \
                 tc.tile_pool(name="psE", bufs=2, space="PSUM") as psE, \
                 tc.tile_pool(name="psPam", bufs=1, space="PSUM") as psP:
                for ich in range(ICH):
                    isl = slice(ich * 512, (ich + 1) * 512)
                    xr = xres.tile([P, CT, 512], F32, tag="xr")
                    nc.sync.dma_start(out=xr, in_=xr32[:, :, isl])
                    pam = [psP.tile([P, 512], F32, tag=f"pam{t}", name=f"pam{t}_{ich}")
                           for t in range(CT)]
                    cs = psS.tile([16, 512], F32, tag="cs")  # 16 identical rows
                    pt2 = None
                    for jt in range(JT):
                        ep = psE.tile([P, 512], F32, tag="e")
                        nc.tensor.matmul(
                            ep, k_sb[:, 0:2, jt * P:(jt + 1) * P], q_sb[:, 0:2, isl],
                            start=True, stop=True, perf_mode=DR)
                        if jt % 2 == 0:
                            pt2 = ptp.tile([P, 2, 512], FP8, tag="pt")
                        nc.scalar.activation(pt2[:, jt % 2, :], ep, AF.Exp,
                                             scale=SC, bias=esh_sb[:, 0:1])
                        if jt % 2 == 1:
                            nc.tensor.matmul(cs, ones2[:, 0:2, 0:16], pt2,
                                             start=(jt == 1), stop=(jt == JT - 1),
                                             perf_mode=DR)
                            for ct in range(CT):
                                nc.tensor.matmul(
                                    pam[ct], vT_sb[:, jt - 1:jt + 1, ct * P:(ct + 1) * P],
                                    pt2, start=(jt == 1), stop=(jt == JT - 1),
                                    perf_mode=DR)
                    inv = sst.tile([1, 512], F32, tag="inv")
                    nc.vector.reciprocal(inv, cs[0:1, :])
                    inv2 = sst.tile([1, 512], F32, tag="inv2")
                    nc.vector.tensor_scalar_mul(inv2, inv, g1_sb[0:1, 0:1])
                    bcs = sst.tile([P, 512], F32, tag="bcs")
                    nc.gpsimd.partition_broadcast(bcs, inv2)
                    for ct in range(CT):
                        pams = sst.tile([P, 512], F32, tag="pams")
                        nc.vector.tensor_copy(pams, pam[ct])
                        t1 = sst.tile([P, 512], F32, tag="t1")
                        nc.vector.tensor_mul(t1, pams, bcs)
                        t2 = sst.tile([P, 512], F32, tag="t2")
                        nc.vector.tensor_add(t2, t1, xr[:, ct, :])
                        nc.scalar.activation(sa_sb[:, ct, isl], t2, AF.Tanh,
                                             bias=cb_sb[:, ct:ct + 1])
                        for it in range(ich * 4, ich * 4 + 4):
                            tp = psTr.tile([P, P], BF16, tag="tp")
                            nc.tensor.transpose(
                                tp, sa_sb[:, ct, it * P:(it + 1) * P], ident)
                            nc.vector.tensor_copy(
                                saT_sb[:, it, ct * P:(ct + 1) * P], tp)

        # ---------------- phase C: CAM ----------------
        with tc.tile_pool(name="phC", bufs=1) as phC, \
             tc.tile_pool(name="stg", bufs=3) as stg, \
             tc.tile_pool(name="psT", bufs=3, space="PSUM") as psT, \
             tc.tile_pool(name="psG", bufs=4, space="PSUM") as psG:
            gp_sb = phC.tile([P, CT, C], F32)
            g2_sb = phC.tile([P, CT, C], F32)
            a_sb = phC.tile([P, CT, C], BF16)
            aT_sb = phC.tile([P, CT, C], BF16)
            # pipelined per-ct: Gram -> AllReduce chunk -> softmax -> aT slice
            for ct in range(CT):
                gp = psG.tile([P, C], F32, tag="g")
                for it in range(HN // P):
                    nc.tensor.matmul(gp, saT_sb[:, it, ct * P:(ct + 1) * P],
                                     saT_sb[:, it, :],
                                     start=(it == 0), stop=(it == HN // P - 1))
                nc.scalar.activation(gp_sb[:, ct, :], gp, AF.Copy)
                nc.sync.dma_start(out=g_ins[ct][:], in_=gp_sb[:, ct, :])
                if sim:
                    # timing-only stand-in for the pairwise AllReduce
                    nc.sync.dma_start(out=g_outs[ct][:], in_=g_ins[ct][:])
                else:
                    nc.gpsimd.collective_compute(
                        "AllReduce", mybir.AluOpType.add,
                        replica_groups=[[0, 1], [2, 3], [4, 5], [6, 7]],
                        ins=[g_ins[ct][:].opt()], outs=[g_outs[ct][:].opt()])
                nc.sync.dma_start(out=g2_sb[:, ct, :], in_=g_outs[ct][:])
                m = stg.tile([P, 1], F32, tag="m")
                nc.vector.tensor_reduce(out=m, in_=g2_sb[:, ct, :],
                                        op=mybir.AluOpType.min,
                                        axis=mybir.AxisListType.X)
                msc = stg.tile([P, 1], F32, tag="msc")
                nc.vector.tensor_scalar_mul(msc, m, SN)
                s = stg.tile([P, 1], F32, tag="s")
                e = stg.tile([P, C], F32, tag="ec")
                nc.scalar.activation(e, g2_sb[:, ct, :], AF.Exp,
                                     bias=msc, scale=-SN, accum_out=s)
                invc = stg.tile([P, 1], F32, tag="invc")
                nc.vector.reciprocal(invc, s)
                nc.scalar.activation(a_sb[:, ct, :], e, AF.Identity, scale=invc)
                for dt in range(CT):
                    tp = psT.tile([P, P], BF16, tag="tp")
                    nc.tensor.transpose(tp, a_sb[:, ct, dt * P:(dt + 1) * P], ident)
                    nc.scalar.activation(aT_sb[:, dt, ct * P:(ct + 1) * P], tp, AF.Copy)
            for ct in range(CT):
                for ich in range(ICH):
                    isl = slice(ich * 512, (ich + 1) * 512)
                    cp = psG.tile([P, 512], F32, tag="g")
                    for dt in range(CT):
                        nc.tensor.matmul(cp, aT_sb[:, dt, ct * P:(ct + 1) * P],
                                         sa_sb[:, dt, isl],
                                         start=(dt == 0), stop=(dt == CT - 1))
                    t1 = stg.tile([P, 512], F32, tag="o1")
                    nc.vector.tensor_scalar_mul(t1, cp, gc_sb[:, 0:1])
                    o = stg.tile([P, 512], F32, tag="o2")
                    nc.vector.tensor_add(o, t1, sa_sb[:, ct, isl])
                    nc.sync.dma_start(out=outp[:, ct, isl], in_=o)
    nc.compile()
    return nc


def _get_nc():
    if "nc" not in _CACHE:
        _CACHE["nc"] = _build_bass()
    return _CACHE["nc"]


def _part(a2d, nt, dtype=np.float32):
    """[nt*128, F] -> [128, nt, F] contiguous (partition-major tiles)."""
    f = a2d.shape[1]
    return np.ascontiguousarray(
        a2d.reshape(nt, P, f).transpose(1, 0, 2).astype(dtype))


def _in_maps(x, wq, bq, wk, bk, wv, bv, gamma_pam, gamma_cam):
    gp = float(np.asarray(gamma_pam).reshape(-1)[0])
    gc = float(np.asarray(gamma_cam).reshape(-1)[0])
    wq_a = _part(np.asarray(wq, np.float32).T, CT, NPBF)
    wk_a = _part(np.asarray(wk, np.float32).T, CT, NPBF)
    wv_a = _part(np.asarray(wv, np.float32).T, CT, NPBF)
    bq_a = np.ascontiguousarray(np.asarray(bq, np.float32).reshape(QT, P).T)
    bk_a = np.ascontiguousarray(np.asarray(bk, np.float32).reshape(QT, P).T)
    cb_a = np.ascontiguousarray(
        (gp * np.asarray(bv, np.float32) / N).reshape(CT, P).T)
    g1_a = np.full((1, 1), gp / N, np.float32)
    gc_a = np.full((P, 1), gc / C, np.float32)
    maps = []
    for core in range(8):
        b, h = core // 2, core % 2
        xr = np.asarray(x, np.float32)[b].reshape(C, N)
        xperm = np.concatenate(
            [xr[:, h * HN:(h + 1) * HN], xr[:, (1 - h) * HN:(2 - h) * HN]], axis=1)
        maps.append({
            "xb": _part(xperm, CT, NPBF),
            "xr32": _part(xperm[:, :HN], CT),
            "wqT": wq_a, "wkT": wk_a, "wvT": wv_a,
            "bq": bq_a, "bk": bk_a, "cb": cb_a, "g1": g1_a, "gcv": gc_a,
        })
    return maps


def _run(in_maps, **kw):
    return run_bass_kernel_spmd(_get_nc(), in_maps, list(range(8)), **kw)


def kernel(**inputs) -> np.ndarray:
    maps = _in_maps(**inputs)
    res = _run(maps).results
    out = np.zeros((B, C, N), np.float32)
    for core in range(8):
        b, h = core // 2, core % 2
        o = np.asarray(res[core]["out"])  # [128, CT, HN]
        out[b][:, h * HN:(h + 1) * HN] = o.transpose(1, 0, 2).reshape(C, HN)
    return out.reshape(B, C, H, W)
